# revision 27
# baseline (speedup 1.0000x reference)
"""Farthest-point sampling (FPS) Bass kernel for Trainium2, 8 NeuronCores.

Input  x: [32, 131072, 3] f32. Output: [32, 2048, 3] f32 (the sampled points,
matching the jax reference's float32 op order; first-occurrence argmax ties).

Sharding: data-parallel over batch. 4 clouds per core; inside a core the 4
clouds are fused into the 128 SBUF partitions (32 partitions per cloud,
4096 columns). Per FPS iteration (serial chain of 2047):
  P1 (DVE custom) a01   = (x0-c0)^2 + (x1-c1)^2
  P2 (DVE custom) s     = (x2-c2)^2 + a01
  P3 (DVE custom) dists = min(dists, s); m[p] = max_col(dists[p])
  P4 max_index    idx8[p] = first col where dists[p]==m[p]
  tail: cross-partition winner per cloud (PE transpose + small DVE ops,
        exact first-occurrence tie-break via encoded flat index), indirect
        DMA gather of the winner's coords (-> next centroid + output row).

Near-ties between the device's plainly-rounded f32 arithmetic and the
reference's (possibly FMA-contracted) arithmetic can swap adjacent picks;
measured effect on this input is a single 2-point swap (rel_norm 5.9e-3),
within the 2e-2 gate, so no detector/fallback is carried.
"""
import os
import numpy as np

import concourse.bass as bass
import concourse.mybir as mybir
import concourse.tile as tile
from concourse import dve_ops
from concourse.bass_utils import run_bass_kernel_spmd
from concourse.dve_spec import Spec, Src0, Src1, C0, C1, minn, maxx, sq, lower
from concourse.dve_uop import DveOpSpec

# ----------------------------------------------------------------------------
# problem constants (hardcoded per task contract)
B, N, K = 32, 131072, 2048
NCORES = 8
BPC = B // NCORES          # clouds per core = 4
PPC = 128 // BPC           # partitions per cloud = 32
COLS = N // PPC            # 4096
BIG = float(2 ** 21)       # > max flat index per core cloud; f32-exact offset
FP = mybir.dt.float32

# ----------------------------------------------------------------------------
# custom DVE ops


def _mk_op(name, spec):
    shas = {}
    for ver in ("v3", "v4"):
        try:
            uops = lower(spec, ver=ver)
            shas[ver] = DveOpSpec(name=name, opcode=0, uops=uops, rd1_en=True).sha(ver)
        except Exception:
            pass
    return dve_ops.DveOp(name, spec, False, shas)


def _ref_sqsq(in0, in1, s0, s1, imm2):
    a = (in0.astype(np.float32) - s0) * (in0.astype(np.float32) - s0)
    b = (in1.astype(np.float32) - s1) * (in1.astype(np.float32) - s1)
    return (a + b).astype(np.float32)


def _ref_sqacc(in0, in1, s0, s1, imm2):
    a = (in0.astype(np.float32) - s0) * (in0.astype(np.float32) - s0)
    return (a + in1).astype(np.float32)


def _ref_minmax(in0, in1, s0, s1, imm2):
    b = np.minimum(in0.astype(np.float32), in1.astype(np.float32))
    return b, b.reshape(b.shape[0], -1).max(axis=-1, keepdims=True)


SQSQ_ANT = _mk_op("SQSQ_ANT", Spec(body=sq(Src0 - C0) + sq(Src1 - C1), reference=_ref_sqsq))
SQACC_ANT = _mk_op("SQACC_ANT", Spec(body=sq(Src0 - C0) + Src1, reference=_ref_sqacc))
MINMAX_ANT = _mk_op("MINMAX_ANT", Spec(body=minn(Src0, Src1), accum=maxx, reference=_ref_minmax))


def _register_ops():
    for op in (SQSQ_ANT, SQACC_ANT, MINMAX_ANT):
        if op.name in dve_ops._SUB_OPCODE_FOR_NAME:
            continue
        dve_ops.OPS.append(op)
        dve_ops._SUB_OPCODE_FOR_NAME[op.name] = max(dve_ops._SUB_OPCODE_FOR_NAME.values()) + 1
        dve_ops.CUSTOM_DVE_SPECS[op.name] = op.spec
    assert max(dve_ops._SUB_OPCODE_FOR_NAME.values()) < 0x20


_register_ops()

# ----------------------------------------------------------------------------
# pre-walrus fixups for this container's toolchain


def _finalize_for_compile(nc):
    """1. codegen_inst_isa_subclasses: fill .instr bytes of raw-ISA insts
    (custom DVE etc.), else walrus fails with "ISA wrong length".
    2. split multi-wait sync_info: this walrus accepts at most ONE sync wait
    per instruction; hoist extras onto preceding single-wait NOPs."""
    nc.thaw()
    mybir.codegen_inst_isa_subclasses(nc)
    ctr = 0
    for func in nc.m.functions:
        for bb in func.blocks:
            new_list = []
            changed = False
            for inst in bb.instructions:
                si = inst.sync_info
                if si is not None and len(si.on_wait) > 1:
                    waits = list(si.on_wait)
                    for w in waits[:-1]:
                        ctr += 1
                        new_list.append(mybir.InstNoOp(
                            name=f"waitsplit-{id(nc)}-{ctr}",
                            engine=inst.engine,
                            sync_info=mybir.SyncInfo(on_wait=[w], on_update=[]),
                            ins=[], outs=[]))
                    inst.sync_info = mybir.SyncInfo(
                        on_wait=[waits[-1]], on_update=list(si.on_update))
                    changed = True
                new_list.append(inst)
            if changed:
                bb.instructions[:] = new_list
    nc.freeze()


def _bcast_inner(ap, reps):
    """[1, C] AP -> [1, C, reps] read-AP with 0-step inner broadcast dim."""
    return bass.AP(tensor=ap.tensor, offset=ap.offset,
                   ap=[ap.ap[0], ap.ap[1], [0, reps]])


# ----------------------------------------------------------------------------
# kernel build


def _build(unroll: int, finalize: bool = True):
    nc = bass.Bass(trn_type="TRN2")
    x_in = nc.dram_tensor("x", [BPC, N, 3], FP, kind="ExternalInput")
    out = nc.dram_tensor("out", [BPC, K, 3], FP, kind="ExternalOutput")
    x_flat = x_in.rearrange("c n k -> (c n) k")      # [BPC*N, 3] gather table
    out_flat = out.rearrange("c t k -> (c t) k")     # [BPC*K, 3] scatter table

    # host-side constant tensors
    ident_np = np.eye(128, dtype=np.float32)
    p_local = (np.arange(128) % PPC).astype(np.float64)
    cloud_of = (np.arange(128) // PPC).astype(np.float64)
    # global flat row index base per partition (incl. cloud offset) + BIG
    rowbaseB_np = (p_local * COLS + cloud_of * N + BIG).reshape(128, 1).astype(np.float32)
    initidx_np = ((np.arange(128) // PPC) * N).astype(np.int32).reshape(128, 1)
    outcnt0_np = (np.arange(BPC, dtype=np.int32) * K).reshape(BPC, 1)
    outcap_np = (np.arange(BPC, dtype=np.int32) * K + (K - 1)).reshape(BPC, 1)
    grep4_np = (np.arange(128) // PPC == np.arange(BPC)[:, None]).astype(np.float32)  # [BPC,128]

    with tile.TileContext(nc) as tc:
        with tc.tile_pool(name="big", bufs=1) as bigp, \
             tc.tile_pool(name="small", bufs=1) as smp, \
             tc.tile_pool(name="ps", bufs=1, space="PSUM") as psp:
            x0 = bigp.tile([128, COLS], FP, tag="x0")
            x1 = bigp.tile([128, COLS], FP, tag="x1")
            x2 = bigp.tile([128, COLS], FP, tag="x2")
            dists = bigp.tile([128, COLS], FP, tag="dists")
            a01 = bigp.tile([128, COLS], FP, tag="a01")
            s = bigp.tile([128, COLS], FP, tag="s")

            ident = smp.tile([128, 128], FP, tag="ident")
            rowbaseB = smp.tile([128, 1], FP, tag="rowbaseB")
            bias = smp.tile([128, 3], FP, tag="bias")
            mc = smp.tile([128, 2], FP, tag="mc")
            idx8 = smp.tile([128, 8], mybir.dt.uint32, tag="idx8")
            M4 = smp.tile([1, BPC], FP, tag="M4")
            eq = smp.tile([1, 128], FP, tag="eq")
            selv = smp.tile([1, 128], FP, tag="selv")
            win4 = smp.tile([1, BPC], FP, tag="win4")
            idx4 = smp.tile([BPC, 1], mybir.dt.int32, tag="idx4")
            bias4 = smp.tile([BPC, 3], FP, tag="bias4")
            initidx = smp.tile([128, 1], mybir.dt.int32, tag="initidx")
            outcnt = smp.tile([BPC, 1], mybir.dt.int32, tag="outcnt")
            outcap = smp.tile([BPC, 1], mybir.dt.int32, tag="outcap")
            grep4 = smp.tile([BPC, 128], FP, tag="grep4")

            mT = psp.tile([1, 128], FP, tag="mT", space="PSUM")
            candT = psp.tile([1, 128], FP, tag="candT", space="PSUM")
            gidxT = psp.tile([BPC, 1], FP, tag="gidxT", space="PSUM")
            biasP = psp.tile([128, 3], FP, tag="biasP", space="PSUM")

            # ---- init ----
            for cst, arr in ((ident, ident_np), (rowbaseB, rowbaseB_np),
                             (initidx, initidx_np), (outcnt, outcnt0_np),
                             (outcap, outcap_np), (grep4, grep4_np)):
                dram = nc.inline_tensor(arr, name=f"const_{cst.tensor.name}")
                nc.sync.dma_start(out=cst[:], in_=dram[:, :])

            NCHUNK = 4
            CCH = COLS // NCHUNK
            for c in range(BPC):
                rows = slice(PPC * c, PPC * c + PPC)
                for j, xt in enumerate((x0, x1, x2)):
                    src = x_in[c, :, j].rearrange("(p n) -> p n", p=PPC)
                    for ch in range(NCHUNK):
                        cols = slice(CCH * ch, CCH * ch + CCH)
                        nc.sync.dma_start(out=xt[rows, cols], in_=src[:, cols])
            nc.vector.memset(dists[:], 3.4e38)

            # initial centroid = point 0 of each cloud; also output row t=0
            nc.gpsimd.indirect_dma_start(
                out=bias[:], out_offset=None, in_=x_flat[:, :],
                in_offset=bass.IndirectOffsetOnAxis(ap=initidx[:, 0:1], axis=0))
            nc.gpsimd.indirect_dma_start(
                out=out_flat[:, :],
                out_offset=bass.IndirectOffsetOnAxis(ap=outcnt[:, 0:1], axis=0),
                in_=bias[0:128:PPC, :], in_offset=None)

            def body(csrc):
                # distance + min-update + per-partition max; centroid read
                # from SBUF (first iter) or straight from PSUM (biasP).
                nc.vector._custom_dve(SQSQ_ANT, out=a01[:], in0=x0[:], in1=x1[:],
                                      s0=csrc[:, 0:1], s1=csrc[:, 1:2])
                nc.vector._custom_dve(SQACC_ANT, out=s[:], in0=x2[:], in1=a01[:],
                                      s0=csrc[:, 2:3])
                nc.vector._custom_dve(MINMAX_ANT, out=dists[:], in0=dists[:],
                                      in1=s[:], accum_out=mc[:, 0:1])
                # while DVE scans max_index: PE transposes the per-partition
                # maxima (for eq), and Pool does the per-cloud max as 4
                # partition-axis reductions straight from SBUF — both off the
                # DVE critical path.
                nc.tensor.transpose(out=mT[:], in_=mc[:, 0:1], identity=ident[:])
                for c in range(BPC):
                    nc.gpsimd.tensor_reduce(
                        M4[0:1, c:c + 1], mc[PPC * c:PPC * c + PPC, 0:1],
                        axis=mybir.AxisListType.C, op=mybir.AluOpType.max)
                # per-partition first-occurrence argmax col
                nc.vector.max_index(idx8[:], mc[:, 0:1].to_broadcast([128, 8]),
                                    dists[:])
                # cast + candidate = BIG + global flat row idx (incl cloud base)
                nc.vector.tensor_scalar(mc[:, 1:2], idx8[:, 0:1], rowbaseB[:, 0:1],
                                        None, op0=mybir.AluOpType.add)
                nc.tensor.transpose(out=candT[:], in_=mc[:, 1:2], identity=ident[:])
                nc.vector.tensor_tensor(
                    out=eq[:].rearrange("o (c p) -> o c p", c=BPC),
                    in0=mT[0:1, :].rearrange("o (c p) -> o c p", c=BPC),
                    in1=_bcast_inner(M4[:], PPC),
                    op=mybir.AluOpType.is_equal)
                nc.vector.scalar_tensor_tensor(
                    out=selv[:], in0=eq[:], scalar=-BIG, in1=candT[0:1, :],
                    op0=mybir.AluOpType.mult, op1=mybir.AluOpType.add)
                nc.vector.tensor_reduce(
                    win4[:], selv[:].rearrange("o (c p) -> o c p", c=BPC),
                    axis=mybir.AxisListType.X, op=mybir.AluOpType.min)
                nc.tensor.transpose(out=gidxT[:], in_=win4[:], identity=ident[0:1, 0:1])
                nc.vector.tensor_copy(idx4[:], gidxT[:])              # f32 -> i32
                # 4-row winner gather -> PE broadcast into biasP + output row
                nc.gpsimd.indirect_dma_start(
                    out=bias4[:], out_offset=None, in_=x_flat[:, :],
                    in_offset=bass.IndirectOffsetOnAxis(ap=idx4[:, 0:1], axis=0))
                nc.tensor.matmul(biasP[:], lhsT=grep4[:], rhs=bias4[:],
                                 start=True, stop=True)
                # outcnt = min(outcnt + 1, per-cloud cap) on DVE (Pool has no
                # min). The clamp is a no-op for the real 2047-iteration build
                # and keeps long timing builds (FPS_BUILD_ITERS > 2047) from
                # scattering out of bounds.
                nc.vector.tensor_scalar_add(outcnt[:], outcnt[:], 1)
                nc.vector.tensor_tensor(out=outcnt[:], in0=outcnt[:],
                                        in1=outcap[:, 0:1],
                                        op=mybir.AluOpType.min)
                nc.gpsimd.indirect_dma_start(
                    out=out_flat[:, :],
                    out_offset=bass.IndirectOffsetOnAxis(ap=outcnt[:, 0:1], axis=0),
                    in_=bias4[:, :], in_offset=None)

            n_iter = int(os.environ.get("FPS_BUILD_ITERS", str(K - 1)))
            # first body reads the DMA'd initial centroid from SBUF; all
            # later bodies read the previous winner straight from PSUM.
            body(bias)
            n_rest = n_iter - 1
            if unroll >= n_rest:
                for _ in range(n_rest):
                    body(biasP)
            else:
                n_loop = n_rest // unroll
                rem = n_rest - n_loop * unroll
                with tc.For_i(0, n_loop, 1):
                    for _ in range(unroll):
                        body(biasP)
                for _ in range(rem):
                    body(biasP)

    if finalize:
        _finalize_for_compile(nc)
    return nc


_NC_CACHE = {}


def _get_nc(unroll):
    if unroll not in _NC_CACHE:
        _NC_CACHE[unroll] = _build(unroll)
    return _NC_CACHE[unroll]


def kernel(x: np.ndarray) -> np.ndarray:
    assert x.shape == (B, N, 3) and x.dtype == np.float32, (x.shape, x.dtype)
    unroll = int(os.environ.get("FPS_UNROLL", "8"))
    nc = _get_nc(unroll)
    in_maps = [{"x": np.ascontiguousarray(x[c * BPC:(c + 1) * BPC])}
               for c in range(NCORES)]
    res = run_bass_kernel_spmd(nc, in_maps, core_ids=list(range(NCORES)))
    if res.exec_time_ns is not None:
        print(f"HW exec time: {res.exec_time_ns} ns")
    y = np.concatenate([r["out"] for r in res.results], axis=0)
    return y


# revision 36
# speedup vs baseline: 1.0326x; 1.0326x over previous
"""Farthest-point sampling (FPS) Bass kernel for Trainium2, 8 NeuronCores.

Input  x: [32, 131072, 3] f32. Output: [32, 2048, 3] f32 (the sampled points,
matching the jax reference's float32 op order; first-occurrence argmax ties).

Sharding: data-parallel over batch. 4 clouds per core; inside a core the 4
clouds are fused into the 128 SBUF partitions (32 partitions per cloud,
4096 columns). Per FPS iteration (serial chain of 2047):
  P1 (DVE custom) a01   = (x0-c0)^2 + (x1-c1)^2
  P2 (DVE custom) s     = (x2-c2)^2 + a01
  P3 (DVE custom) dists = min(dists, s); m[p] = max_col(dists[p])
  P4 max_index    idx8[p] = first col where dists[p]==m[p]
  tail: cross-partition winner per cloud (PE transpose + small DVE ops,
        exact first-occurrence tie-break via encoded flat index), indirect
        DMA gather of the winner's coords (-> next centroid + output row).

Near-ties between the device's plainly-rounded f32 arithmetic and the
reference's (possibly FMA-contracted) arithmetic can swap adjacent picks;
measured effect on this input is a single 2-point swap (rel_norm 5.9e-3),
within the 2e-2 gate, so no detector/fallback is carried.
"""
import os
import numpy as np

import concourse.bass as bass
import concourse.mybir as mybir
import concourse.tile as tile
from concourse import dve_ops
from concourse.bass_utils import run_bass_kernel_spmd
from concourse.dve_spec import (Spec, Src0, Src1, C0, C1, C2, Zero, One,
                                minn, maxx, sq, eq, select, scan, AluOp, lower)
from concourse.dve_uop import DveOpSpec

# ----------------------------------------------------------------------------
# problem constants (hardcoded per task contract)
B, N, K = 32, 131072, 2048
NCORES = 8
BPC = B // NCORES          # clouds per core = 4
PPC = 128 // BPC           # partitions per cloud = 32
COLS = N // PPC            # 4096
BIG = float(2 ** 21)       # > max flat index per core cloud; f32-exact offset
FP = mybir.dt.float32

# ----------------------------------------------------------------------------
# custom DVE ops


def _mk_op(name, spec):
    shas = {}
    for ver in ("v3", "v4"):
        try:
            uops = lower(spec, ver=ver)
            shas[ver] = DveOpSpec(name=name, opcode=0, uops=uops, rd1_en=True).sha(ver)
        except Exception:
            pass
    return dve_ops.DveOp(name, spec, False, shas)


def _ref_sqsq(in0, in1, s0, s1, imm2):
    a = (in0.astype(np.float32) - s0) * (in0.astype(np.float32) - s0)
    b = (in1.astype(np.float32) - s1) * (in1.astype(np.float32) - s1)
    return (a + b).astype(np.float32)


def _ref_sqacc(in0, in1, s0, s1, imm2):
    a = (in0.astype(np.float32) - s0) * (in0.astype(np.float32) - s0)
    return (a + in1).astype(np.float32)


def _ref_minmax(in0, in1, s0, s1, imm2):
    b = np.minimum(in0.astype(np.float32), in1.astype(np.float32))
    return b, b.reshape(b.shape[0], -1).max(axis=-1, keepdims=True)


def _ref_pairidx(in0, in1, s0, s1, imm2):
    # in0 = even cols of dists, in1 = odd cols; s0 = per-partition max;
    # out_k = NEGATED first-occurrence flat col of the max within pair k
    # (or -3.4e38); accum = max over pairs = -(first argmax col).
    e0 = in0.astype(np.float32) == s0
    e1 = in1.astype(np.float32) == s0
    k = np.arange(in0.shape[-1], dtype=np.float32)
    odd = -(2.0 * k + 1.0)
    out = np.where(e0, odd + 1.0,
                   np.where(e1, odd, np.float32(-3.4e38))).astype(np.float32)
    return out, out.reshape(out.shape[0], -1).max(axis=-1, keepdims=True)


SQSQ_ANT = _mk_op("SQSQ_ANT", Spec(body=sq(Src0 - C0) + sq(Src1 - C1), reference=_ref_sqsq))
SQACC_ANT = _mk_op("SQACC_ANT", Spec(body=sq(Src0 - C0) + Src1, reference=_ref_sqacc))
MINMAX_ANT = _mk_op("MINMAX_ANT", Spec(body=minn(Src0, Src1), accum=maxx, reference=_ref_minmax))
# two-ports-wide first-occurrence argmax: reads dists as (even, odd) column
# pairs -> 2 elements/cycle; emits per-pair "flat col of the max or sentinel",
# accum-min folds to the per-partition first argmax column.
from concourse.dve_spec import MaxNeg
_sc_nodd = scan(AluOp.ADD, C2, init=One)   # -(2k+1) at pair k (imm2=-2)
PAIRIDX_ANT = _mk_op("PAIRIDX_ANT", Spec(
    body=select(eq(Src0, C0), _sc_nodd + One,
                select(eq(Src1, C0), _sc_nodd, MaxNeg)),
    accum=maxx,
    reference=_ref_pairidx))


def _register_ops():
    for op in (SQSQ_ANT, SQACC_ANT, MINMAX_ANT, PAIRIDX_ANT):
        if op.name in dve_ops._SUB_OPCODE_FOR_NAME:
            continue
        dve_ops.OPS.append(op)
        dve_ops._SUB_OPCODE_FOR_NAME[op.name] = max(dve_ops._SUB_OPCODE_FOR_NAME.values()) + 1
        dve_ops.CUSTOM_DVE_SPECS[op.name] = op.spec
    assert max(dve_ops._SUB_OPCODE_FOR_NAME.values()) < 0x20


_register_ops()

# ----------------------------------------------------------------------------
# pre-walrus fixups for this container's toolchain


def _finalize_for_compile(nc):
    """1. codegen_inst_isa_subclasses: fill .instr bytes of raw-ISA insts
    (custom DVE etc.), else walrus fails with "ISA wrong length".
    2. split multi-wait sync_info: this walrus accepts at most ONE sync wait
    per instruction; hoist extras onto preceding single-wait NOPs."""
    nc.thaw()
    mybir.codegen_inst_isa_subclasses(nc)
    ctr = 0
    for func in nc.m.functions:
        for bb in func.blocks:
            new_list = []
            changed = False
            for inst in bb.instructions:
                si = inst.sync_info
                if si is not None and len(si.on_wait) > 1:
                    waits = list(si.on_wait)
                    for w in waits[:-1]:
                        ctr += 1
                        new_list.append(mybir.InstNoOp(
                            name=f"waitsplit-{id(nc)}-{ctr}",
                            engine=inst.engine,
                            sync_info=mybir.SyncInfo(on_wait=[w], on_update=[]),
                            ins=[], outs=[]))
                    inst.sync_info = mybir.SyncInfo(
                        on_wait=[waits[-1]], on_update=list(si.on_update))
                    changed = True
                new_list.append(inst)
            if changed:
                bb.instructions[:] = new_list
    nc.freeze()


def _bcast_inner(ap, reps):
    """[1, C] AP -> [1, C, reps] read-AP with 0-step inner broadcast dim."""
    return bass.AP(tensor=ap.tensor, offset=ap.offset,
                   ap=[ap.ap[0], ap.ap[1], [0, reps]])


# ----------------------------------------------------------------------------
# kernel build


def _build(unroll: int, finalize: bool = True):
    nc = bass.Bass(trn_type="TRN2")
    x_in = nc.dram_tensor("x", [BPC, N, 3], FP, kind="ExternalInput")
    out = nc.dram_tensor("out", [BPC, K, 3], FP, kind="ExternalOutput")
    x_flat = x_in.rearrange("c n k -> (c n) k")      # [BPC*N, 3] gather table
    out_flat = out.rearrange("c t k -> (c t) k")     # [BPC*K, 3] scatter table

    # host-side constant tensors
    ident_np = np.eye(128, dtype=np.float32)
    p_local = (np.arange(128) % PPC).astype(np.float64)
    cloud_of = (np.arange(128) // PPC).astype(np.float64)
    # global flat row index base per partition (incl. cloud offset) + BIG
    rowbaseB_np = (p_local * COLS + cloud_of * N + BIG).reshape(128, 1).astype(np.float32)
    initidx_np = ((np.arange(128) // PPC) * N).astype(np.int32).reshape(128, 1)
    outcnt0_np = (np.arange(BPC, dtype=np.int32) * K).reshape(BPC, 1)
    outcap_np = (np.arange(BPC, dtype=np.int32) * K + (K - 1)).reshape(BPC, 1)
    grep4_np = (np.arange(128) // PPC == np.arange(BPC)[:, None]).astype(np.float32)  # [BPC,128]

    with tile.TileContext(nc) as tc:
        with tc.tile_pool(name="big", bufs=1) as bigp, \
             tc.tile_pool(name="small", bufs=1) as smp, \
             tc.tile_pool(name="ps", bufs=1, space="PSUM") as psp:
            x0 = bigp.tile([128, COLS], FP, tag="x0")
            x1 = bigp.tile([128, COLS], FP, tag="x1")
            x2 = bigp.tile([128, COLS], FP, tag="x2")
            dists = bigp.tile([128, COLS], FP, tag="dists")
            a01 = bigp.tile([128, COLS], FP, tag="a01")
            s = bigp.tile([128, COLS], FP, tag="s")

            ident = smp.tile([128, 128], FP, tag="ident")
            rowbaseB = smp.tile([128, 1], FP, tag="rowbaseB")
            bias = smp.tile([128, 3], FP, tag="bias")
            mc = smp.tile([128, 2], FP, tag="mc")
            idxf = smp.tile([128, 1], FP, tag="idxf")
            M4 = smp.tile([1, BPC], FP, tag="M4")
            eq = smp.tile([1, 128], FP, tag="eq")
            selv = smp.tile([1, 128], FP, tag="selv")
            win4 = smp.tile([1, BPC], FP, tag="win4")
            idx4 = smp.tile([BPC, 1], mybir.dt.int32, tag="idx4")
            bias4 = smp.tile([BPC, 3], FP, tag="bias4")
            initidx = smp.tile([128, 1], mybir.dt.int32, tag="initidx")
            outcnt = smp.tile([BPC, 1], mybir.dt.int32, tag="outcnt")
            outcap = smp.tile([BPC, 1], mybir.dt.int32, tag="outcap")
            grep4 = smp.tile([BPC, 128], FP, tag="grep4")

            mT = psp.tile([1, 128], FP, tag="mT", space="PSUM")
            candT = psp.tile([1, 128], FP, tag="candT", space="PSUM")
            gidxT = psp.tile([BPC, 1], FP, tag="gidxT", space="PSUM")
            biasP = psp.tile([128, 3], FP, tag="biasP", space="PSUM")

            # ---- init ----
            for cst, arr in ((ident, ident_np), (rowbaseB, rowbaseB_np),
                             (initidx, initidx_np), (outcnt, outcnt0_np),
                             (outcap, outcap_np), (grep4, grep4_np)):
                dram = nc.inline_tensor(arr, name=f"const_{cst.tensor.name}")
                nc.sync.dma_start(out=cst[:], in_=dram[:, :])

            NCHUNK = 4
            CCH = COLS // NCHUNK
            for c in range(BPC):
                rows = slice(PPC * c, PPC * c + PPC)
                for j, xt in enumerate((x0, x1, x2)):
                    src = x_in[c, :, j].rearrange("(p n) -> p n", p=PPC)
                    for ch in range(NCHUNK):
                        cols = slice(CCH * ch, CCH * ch + CCH)
                        nc.sync.dma_start(out=xt[rows, cols], in_=src[:, cols])
            nc.vector.memset(dists[:], 3.4e38)

            # initial centroid = point 0 of each cloud; also output row t=0
            nc.gpsimd.indirect_dma_start(
                out=bias[:], out_offset=None, in_=x_flat[:, :],
                in_offset=bass.IndirectOffsetOnAxis(ap=initidx[:, 0:1], axis=0))
            nc.gpsimd.indirect_dma_start(
                out=out_flat[:, :],
                out_offset=bass.IndirectOffsetOnAxis(ap=outcnt[:, 0:1], axis=0),
                in_=bias[0:128:PPC, :], in_offset=None)

            def body(csrc):
                # distance + min-update + per-partition max; centroid read
                # from SBUF (first iter) or straight from PSUM (biasP).
                nc.vector._custom_dve(SQSQ_ANT, out=a01[:], in0=x0[:], in1=x1[:],
                                      s0=csrc[:, 0:1], s1=csrc[:, 1:2])
                nc.vector._custom_dve(SQACC_ANT, out=s[:], in0=x2[:], in1=a01[:],
                                      s0=csrc[:, 2:3])
                nc.vector._custom_dve(MINMAX_ANT, out=dists[:], in0=dists[:],
                                      in1=s[:], accum_out=mc[:, 0:1])
                # while DVE scans max_index: PE transposes the per-partition
                # maxima (for eq), and Pool does the per-cloud max as 4
                # partition-axis reductions straight from SBUF — both off the
                # DVE critical path.
                nc.tensor.transpose(out=mT[:], in_=mc[:, 0:1], identity=ident[:])
                for c in range(BPC):
                    nc.gpsimd.tensor_reduce(
                        M4[0:1, c:c + 1], mc[PPC * c:PPC * c + PPC, 0:1],
                        axis=mybir.AxisListType.C, op=mybir.AluOpType.max)
                # per-partition first-occurrence argmax col, 2 cols/cycle:
                # even cols on port 0, odd cols on port 1 (s is dead here,
                # reuse its first half as the throwaway per-pair output).
                nc.vector._custom_dve(
                    PAIRIDX_ANT, out=s[:, 0:COLS // 2],
                    in0=dists[:, 0:COLS:2], in1=dists[:, 1:COLS:2],
                    s0=mc[:, 0:1], imm2=-2.0,
                    accum_out=idxf[:, 0:1])
                # candidate = BIG + global flat row idx (incl cloud base);
                # idxf holds the NEGATED column, so flip sign while adding.
                nc.vector.scalar_tensor_tensor(
                    out=mc[:, 1:2], in0=idxf[:, 0:1], scalar=-1.0,
                    in1=rowbaseB[:, 0:1],
                    op0=mybir.AluOpType.mult, op1=mybir.AluOpType.add)
                nc.tensor.transpose(out=candT[:], in_=mc[:, 1:2], identity=ident[:])
                nc.vector.tensor_tensor(
                    out=eq[:].rearrange("o (c p) -> o c p", c=BPC),
                    in0=mT[0:1, :].rearrange("o (c p) -> o c p", c=BPC),
                    in1=_bcast_inner(M4[:], PPC),
                    op=mybir.AluOpType.is_equal)
                nc.vector.scalar_tensor_tensor(
                    out=selv[:], in0=eq[:], scalar=-BIG, in1=candT[0:1, :],
                    op0=mybir.AluOpType.mult, op1=mybir.AluOpType.add)
                nc.vector.tensor_reduce(
                    win4[:], selv[:].rearrange("o (c p) -> o c p", c=BPC),
                    axis=mybir.AxisListType.X, op=mybir.AluOpType.min)
                nc.tensor.transpose(out=gidxT[:], in_=win4[:], identity=ident[0:1, 0:1])
                nc.vector.tensor_copy(idx4[:], gidxT[:])              # f32 -> i32
                # 4-row winner gather -> PE broadcast into biasP + output row
                nc.gpsimd.indirect_dma_start(
                    out=bias4[:], out_offset=None, in_=x_flat[:, :],
                    in_offset=bass.IndirectOffsetOnAxis(ap=idx4[:, 0:1], axis=0))
                nc.tensor.matmul(biasP[:], lhsT=grep4[:], rhs=bias4[:],
                                 start=True, stop=True)
                # outcnt = min(outcnt + 1, per-cloud cap) on DVE (Pool has no
                # min). The clamp is a no-op for the real 2047-iteration build
                # and keeps long timing builds (FPS_BUILD_ITERS > 2047) from
                # scattering out of bounds.
                nc.vector.tensor_scalar_add(outcnt[:], outcnt[:], 1)
                nc.vector.tensor_tensor(out=outcnt[:], in0=outcnt[:],
                                        in1=outcap[:, 0:1],
                                        op=mybir.AluOpType.min)
                nc.gpsimd.indirect_dma_start(
                    out=out_flat[:, :],
                    out_offset=bass.IndirectOffsetOnAxis(ap=outcnt[:, 0:1], axis=0),
                    in_=bias4[:, :], in_offset=None)

            n_iter = int(os.environ.get("FPS_BUILD_ITERS", str(K - 1)))
            # first body reads the DMA'd initial centroid from SBUF; all
            # later bodies read the previous winner straight from PSUM.
            body(bias)
            n_rest = n_iter - 1
            if unroll >= n_rest:
                for _ in range(n_rest):
                    body(biasP)
            else:
                n_loop = n_rest // unroll
                rem = n_rest - n_loop * unroll
                with tc.For_i(0, n_loop, 1):
                    for _ in range(unroll):
                        body(biasP)
                for _ in range(rem):
                    body(biasP)

    if finalize:
        _finalize_for_compile(nc)
    return nc


_NC_CACHE = {}


def _get_nc(unroll):
    if unroll not in _NC_CACHE:
        _NC_CACHE[unroll] = _build(unroll)
    return _NC_CACHE[unroll]


def kernel(x: np.ndarray) -> np.ndarray:
    assert x.shape == (B, N, 3) and x.dtype == np.float32, (x.shape, x.dtype)
    unroll = int(os.environ.get("FPS_UNROLL", "8"))
    nc = _get_nc(unroll)
    in_maps = [{"x": np.ascontiguousarray(x[c * BPC:(c + 1) * BPC])}
               for c in range(NCORES)]
    res = run_bass_kernel_spmd(nc, in_maps, core_ids=list(range(NCORES)))
    if res.exec_time_ns is not None:
        print(f"HW exec time: {res.exec_time_ns} ns")
    y = np.concatenate([r["out"] for r in res.results], axis=0)
    return y
